# revision 5
# baseline (speedup 1.0000x reference)
"""Trainium2 Bass kernel for the BiRNN LM problem.

Computation (per step t over SEQ=64):
    emb    = we[tok_t]                       [B=32, E=32]
    hidden = tanh([emb, hidden] @ i2h)       [B=32, H=16]
    out_t  = exp(hidden @ i2o)               [B=32, V=32000]
    out_t /= sum(out_t)                      (global sum over the whole slab)

Sharding: sequence dim across 8 cores, interleaved: core c produces output
steps {c + 8k, k=0..7}.  Each step's normalization sum is fully local to one
core => no collectives.  Every core replicates the (tiny) 64-step hidden
recurrence; per-core step selection is data-driven (an offsets input feeding
an indirect gather), so all cores run one identical SPMD program.
"""

import sys
import numpy as np

sys.path.insert(0, "/opt/trn_rl_repo")

import concourse.bass as bass
import concourse.bacc as bacc
import concourse.mybir as mybir
import concourse.tile as tile
from concourse.bass_utils import run_bass_kernel_spmd

F32 = mybir.dt.float32
F32R = mybir.dt.float32r
I32 = mybir.dt.int32
AF = mybir.ActivationFunctionType

SEQ, B, E, H, V = 64, 32, 32, 16, 32000
NCORES = 8
SPC = SEQ // NCORES          # steps per core
NGROUP = 2                   # groups of 4 steps (4*32 = 128 partitions)
GSTEP = 4
CHUNK = 512                  # matmul free dim (one PSUM bank of f32)
MACRO = 1024                 # ACT exp granularity (2 banks)
PIECE = 4096                 # mul + DMA granularity (4 macros)
VQ = 8192                    # padded vocab quarter (i2o rows 32q..32q+16)
VPAD = 4 * VQ                # 32768


def _macro_grid():
    """[(piece_idx, start_col_in_piece, width), ...] covering real V on the padded grid."""
    out = []
    col = 0
    while col < V:
        piece = col // PIECE
        w = min(MACRO, V - col, (piece + 1) * PIECE - col)
        out.append((piece, col - piece * PIECE, w))
        col += w
    return out


def _piece_widths():
    return [min(PIECE, V - k * PIECE) for k in range((V + PIECE - 1) // PIECE)]


def build():
    nc = bacc.Bacc("TRN2", target_bir_lowering=False, debug=False,
                   num_devices=NCORES)

    tok_d = nc.dram_tensor("tokT", [128, 16], I32, kind="ExternalInput")
    h0_d = nc.dram_tensor("h0T", [H, B], F32, kind="ExternalInput")
    we_d = nc.dram_tensor("we", [V, E], F32, kind="ExternalInput")
    wE_d = nc.dram_tensor("wE", [E, H], F32, kind="ExternalInput")   # i2h[:E]
    wH_d = nc.dram_tensor("wH", [H, H], F32, kind="ExternalInput")   # i2h[E:]
    i2o_d = nc.dram_tensor("i2oQ", [128, VQ], F32R, kind="ExternalInput")
    idn_d = nc.dram_tensor("idn", [128, 128], F32, kind="ExternalInput")
    mask_d = nc.dram_tensor("mask4", [128, 4], F32, kind="ExternalInput")
    maskT_d = nc.dram_tensor("maskT4", [4, 128], F32, kind="ExternalInput")
    hsel_d = nc.dram_tensor("hsel", [128, 1], I32, kind="ExternalInput")

    out_d = nc.dram_tensor("out", [NGROUP, 128, V], F32, kind="ExternalOutput")

    hlo_d = nc.dram_tensor("Hlo", [32 * B, H], F32, kind="Internal")
    hhi_d = nc.dram_tensor("Hhi", [32 * B, H], F32, kind="Internal")

    grid = _macro_grid()
    nmacro = sum(1 for g in grid)          # macros per group (same grid each group)
    pieces = _piece_widths()

    with tile.TileContext(nc) as tc:
        with (
            tc.tile_pool(name="const", bufs=1) as constp,
            tc.tile_pool(name="embg", bufs=3) as embgp,
            tc.tile_pool(name="hchain", bufs=3) as hchp,
            tc.tile_pool(name="grp", bufs=2) as grpp,
            tc.tile_pool(name="slab", bufs=len(pieces) + 1) as slabp,
            tc.tile_pool(name="pmm", bufs=2, space="PSUM") as pmmp,
            tc.tile_pool(name="phc", bufs=1, space="PSUM") as phcp,
            tc.tile_pool(name="pmisc", bufs=1, space="PSUM") as pmiscp,
        ):
            # ---- constants / inputs to SBUF ----
            i2o = constp.tile([128, VQ], F32R)
            nc.sync.dma_start(i2o[:], i2o_d.ap())
            tok = constp.tile([128, 16], I32)
            nc.sync.dma_start(tok[:], tok_d.ap())
            idn = constp.tile([128, 128], F32)
            nc.sync.dma_start(idn[:], idn_d.ap())
            mask4 = constp.tile([128, 4], F32)
            nc.sync.dma_start(mask4[:], mask_d.ap())
            maskT4 = constp.tile([4, 128], F32)
            nc.sync.dma_start(maskT4[:], maskT_d.ap())
            wE = constp.tile([E, H], F32)
            nc.sync.dma_start(wE[:], wE_d.ap())
            wH = constp.tile([H, H], F32)
            nc.sync.dma_start(wH[:], wH_d.ap())
            hsel = constp.tile([128, 1], I32)
            nc.sync.dma_start(hsel[:], hsel_d.ap())
            embT = constp.tile([E, SEQ * B], F32)   # [32, 2048]

            # ---- embedding gather + transpose into embT ----
            for j in range(16):
                eg = embgp.tile([128, E], F32, tag="eg")
                nc.gpsimd.indirect_dma_start(
                    out=eg[:], out_offset=None, in_=we_d.ap(),
                    in_offset=bass.IndirectOffsetOnAxis(ap=tok[:, j:j + 1], axis=0))
                ep = phcp.tile([E, 128], F32, space="PSUM", tag="ptr")
                nc.tensor.transpose(ep[:], eg[:], idn[:])
                nc.vector.tensor_copy(embT[:, 128 * j:128 * (j + 1)], ep[:])

            # ---- recurrence ----
            h_prev = hchp.tile([32, B], F32, tag="hp")
            nc.sync.dma_start(h_prev[0:H, :], h0_d.ap())

            def chain_steps(t0, t1, hdram):
                nonlocal h_prev
                for t in range(t0, t1):
                    hp = phcp.tile([H, B], F32, space="PSUM", tag="hps")
                    nc.tensor.matmul(hp[:], wE[:], embT[:, B * t:B * (t + 1)],
                                     start=True, stop=False)
                    nc.tensor.matmul(hp[:], wH[:], h_prev[0:H, :],
                                     start=False, stop=True)
                    hpad = hchp.tile([32, B], F32, tag="hp")
                    nc.scalar.activation(hpad[0:H, :], hp[:], AF.Tanh)
                    htb = hchp.tile([32, B], F32, tag="htb")
                    nc.vector.transpose(htb[:], hpad[:])
                    r0 = B * (t % 32)
                    nc.sync.dma_start(hdram.ap()[r0:r0 + B, :], htb[:, 0:H])
                    h_prev = hpad

            def group(g, hdram):
                hg = grpp.tile([128, H], F32, tag="hg")
                nc.gpsimd.indirect_dma_start(
                    out=hg[:], out_offset=None, in_=hdram.ap(),
                    in_offset=bass.IndirectOffsetOnAxis(ap=hsel[:, 0:1], axis=0))
                hgp = phcp.tile([H, 128], F32, space="PSUM", tag="ptr")
                nc.tensor.transpose(hgp[:], hg[:], idn[:])
                # replicate onto all four quarter base partitions (matmul
                # requires lhsT and rhs to share a base partition)
                lhsT = grpp.tile([128, 128], F32R, tag="lhsT")
                for q in range(4):
                    nc.vector.tensor_copy(lhsT[32 * q:32 * q + H, :], hgp[:])

                partials = grpp.tile([128, nmacro], F32, tag="part")
                slabs = [slabp.tile([128, w], F32, tag="slab", name=f"slab_{g}_{k}")
                         for k, w in enumerate(pieces)]
                for m, (piece, pcol, w) in enumerate(grid):
                    ps = pmmp.tile([128, MACRO], F32, space="PSUM", tag="mm")
                    for c0 in range(0, w, CHUNK):
                        cw = min(CHUNK, w - c0)
                        gcol = piece * PIECE + pcol + c0
                        q, qcol = gcol // VQ, gcol % VQ
                        nc.tensor.matmul(
                            ps[:, c0:c0 + cw], lhsT[32 * q:32 * q + H, :],
                            i2o[32 * q:32 * q + H, qcol:qcol + cw],
                            start=True, stop=True,
                            tile_position=(32 * q, 0))
                    nc.scalar.activation(
                        slabs[piece][:, pcol:pcol + w], ps[:, 0:w], AF.Exp,
                        accum_out=partials[:, m:m + 1])

                sums_ps = pmiscp.tile([4, nmacro], F32, space="PSUM", tag="misc")
                nc.tensor.matmul(sums_ps[:], mask4[:], partials[:],
                                 start=True, stop=True)
                s4 = grpp.tile([4, 1], F32, tag="s4")
                nc.vector.tensor_reduce(s4[:], sums_ps[:],
                                        axis=mybir.AxisListType.X,
                                        op=mybir.AluOpType.add)
                r4 = grpp.tile([4, 1], F32, tag="r4")
                nc.vector.reciprocal(r4[:], s4[:])
                bc_ps = pmiscp.tile([128, 1], F32, space="PSUM", tag="misc")
                nc.tensor.matmul(bc_ps[:], maskT4[:], r4[:], start=True, stop=True)
                scal = grpp.tile([128, 1], F32, tag="scal")
                nc.scalar.copy(scal[:], bc_ps[:])

                for k, w in enumerate(pieces):
                    nc.vector.tensor_scalar_mul(slabs[k][:], slabs[k][:],
                                                scal[:, 0:1])
                    nc.sync.dma_start(out_d.ap()[g, :, PIECE * k:PIECE * k + w],
                                      slabs[k][:])

            chain_steps(0, 32, hlo_d)
            group(0, hlo_d)
            chain_steps(32, 64, hhi_d)
            group(1, hhi_d)

    nc.compile()
    return nc


_NC_CACHE = None


def _get_nc():
    global _NC_CACHE
    if _NC_CACHE is None:
        _NC_CACHE = build()
    return _NC_CACHE


def _prep_inputs(input_tokens, h0, we, i2h, i2o):
    flat = np.ascontiguousarray(input_tokens, dtype=np.int32).reshape(-1)  # (t,b)
    tokT = np.ascontiguousarray(flat.reshape(16, 128).T)                   # [128,16]
    h0T = np.ascontiguousarray(np.asarray(h0, np.float32).T)               # [16,32]
    we = np.ascontiguousarray(np.asarray(we, np.float32))
    i2h = np.asarray(i2h, np.float32)
    wE = np.ascontiguousarray(i2h[:E, :])
    wH = np.ascontiguousarray(i2h[E:, :])
    i2o = np.asarray(i2o, np.float32)
    i2oQ = np.zeros((128, VQ), np.float32)
    for q in range(4):
        lo = VQ * q
        hi = min(lo + VQ, V)
        i2oQ[32 * q:32 * q + H, 0:hi - lo] = i2o[:, lo:hi]
    idn = np.eye(128, dtype=np.float32)
    mask4 = np.zeros((128, 4), np.float32)
    mask4[np.arange(128), np.arange(128) // 32] = 1.0
    maskT4 = np.ascontiguousarray(mask4.T)
    shared = dict(tokT=tokT, h0T=h0T, we=we, wE=wE, wH=wH, i2oQ=i2oQ,
                  idn=idn, mask4=mask4, maskT4=maskT4)
    in_maps = []
    for c in range(NCORES):
        p = np.arange(128)
        hsel = (B * c + 256 * (p // 32) + (p % 32)).astype(np.int32)[:, None]
        in_maps.append(dict(shared, hsel=np.ascontiguousarray(hsel)))
    return in_maps


def _assemble(results):
    full = np.empty((SEQ, B, V), np.float32)
    for c in range(NCORES):
        o = results[c]["out"].reshape(NGROUP, GSTEP, B, V)
        for g in range(NGROUP):
            for i in range(GSTEP):
                full[c + 32 * g + 8 * i] = o[g, i]
    return full


def run(inputs, trace=False, **kw):
    nc = _get_nc()
    in_maps = _prep_inputs(**inputs)
    res = run_bass_kernel_spmd(nc, in_maps, list(range(NCORES)), trace=trace, **kw)
    return _assemble(res.results), res


def kernel(**inputs):
    out, _ = run(inputs, trace=False)
    return out
